# revision 20
# baseline (speedup 1.0000x reference)
"""CondConv2d Trainium2 kernel.

Math: per-sample conv kernel = routing-weighted sum of 8 expert kernels,
then a 3x3 (pad 1, stride 1) conv per sample, plus a routed bias.

Strategy:
  - Host: compute the per-sample combined kernels w_mix (tiny einsum,
    [64,8]@[8,147456]) laid out as [bs, cin, kh, kw, oc] so each tap's
    [cin, oc] slab is directly the matmul stationary operand (lhsT), and
    b_mix transposed to [oc, bs] for per-partition bias adds. x is
    zero-padded to 58x58 and packed together with w_mix into one tensor so
    each sample needs a single input DMA.
  - Shard data-parallel over batch: 8 samples per NeuronCore x 8 cores.
  - Device (per core, per sample): conv = 9 accumulating PE matmuls per
    8-output-row block (K=cin=128, M=oc=128, N=8*56=448) in float32r
    (full-rate fp32 matmul), ScalarE evicts PSUM->SBUF fused with the
    bias add, DMA out.

Everything is hardcoded for the problem shapes:
  x[64,128,56,56] f32, routing_weight[64,8] f32,
  weight[8,128,128,3,3] f32, bias[8,128] f32 -> out[64,128,56,56] f32.
"""

import os

import numpy as np

N_CORES = 8
BS, CIN, H, W = 64, 128, 56, 56
KEXP, OC = 8, 128
P = BS // N_CORES  # samples per core
RB = 8             # output rows per PSUM block
NBLK = H // RB     # 7 blocks
HP, WP = H + 2, W + 2
XSZ = HP * WP      # padded x free size per partition
WSZ = 3 * 3 * OC   # mixed-weight free size per partition
PKSZ = XSZ + WSZ
# input DMA split: part A = weights + first SPLIT_ROWS padded x rows (enough
# for the first two 8-row output blocks), part B = the rest
SPLIT_ROWS = 26

# matmul input dtype: "f32r" (fp32 data, full-rate float32r matmul),
# "f32" (exact fp32, 4x slower), or "bf16" (casts on host).
MM_DTYPE = "f32r"

_CACHE = {}


def _build_nc(mm_dtype):
    import concourse.bacc as bacc
    import concourse.mybir as mybir
    import concourse.tile as tile

    f32 = mybir.dt.float32
    if mm_dtype == "bf16":
        dt_in = mybir.dt.bfloat16
    elif mm_dtype == "f32r":
        dt_in = mybir.dt.float32r
    else:
        dt_in = f32

    nc = bacc.Bacc()
    xw = nc.dram_tensor("xw", [P, CIN, PKSZ], dt_in, kind="ExternalInput")
    bm = nc.dram_tensor("bm", [OC, P], f32, kind="ExternalInput")
    out = nc.dram_tensor("out", [P, OC, H, W], f32, kind="ExternalOutput")

    taps = [(dy, dx) for dy in range(3) for dx in range(3)]

    with tile.TileContext(nc) as tc:
        with (
            tc.tile_pool(name="xp", bufs=3) as xp,
            tc.tile_pool(name="op", bufs=4) as op,
            tc.tile_pool(name="bp", bufs=1) as bp,
            tc.tile_pool(name="ps", bufs=8, space="PSUM") as psp,
        ):
            bmt = bp.tile([OC, P], f32)
            nc.sync.dma_start(out=bmt, in_=bm[:, :])

            def evict(pst, obt_name, b, r0):
                # PSUM -> SBUF with fused routed-bias add, then DMA out
                obt = op.tile([OC, RB, W], f32, tag="obt", name=obt_name)
                nc.scalar.activation(
                    out=obt,
                    in_=pst[:, :, :],
                    func=mybir.ActivationFunctionType.Identity,
                    bias=bmt[:, b : b + 1],
                    scale=1.0,
                )
                nc.sync.dma_start(out=out[b][:, r0 : r0 + RB, :], in_=obt)

            for b in range(P):
                # packed input layout: [w_mix (WSZ) | padded x (XSZ)]
                xwt = xp.tile([CIN, PKSZ], dt_in)
                if b == 0:
                    # chunked load so block 0 can start as early as possible
                    c0 = WSZ + 10 * WP
                    c1 = WSZ + 26 * WP
                    nc.sync.dma_start(out=xwt[:, :c0], in_=xw[b][:, :c0])
                    nc.sync.dma_start(out=xwt[:, c0:c1], in_=xw[b][:, c0:c1])
                    nc.sync.dma_start(out=xwt[:, c1:], in_=xw[b][:, c1:])
                else:
                    nc.sync.dma_start(out=xwt, in_=xw[b])
                wt = xwt[:, :WSZ].rearrange(
                    "p (kh kw oc) -> p kh kw oc", kh=3, kw=3
                )
                xt = xwt[:, WSZ:].rearrange("p (h w) -> p h w", h=HP)
                pst = [
                    psp.tile([OC, RB, W], f32, tag="pst", name=f"pst{b}_{i}")
                    for i in range(NBLK)
                ]
                if b == 0:
                    # block-outer: compute each block fully as its x rows land
                    for blk in range(NBLK):
                        r0 = blk * RB
                        for it, (dy, dx) in enumerate(taps):
                            nc.tensor.matmul(
                                out=pst[blk][:, :, :],
                                lhsT=wt[:, dy, dx, :],
                                rhs=xt[:, r0 + dy : r0 + dy + RB, dx : dx + W],
                                start=(it == 0),
                                stop=(it == len(taps) - 1),
                                skip_group_check=True,
                            )
                        evict(pst[blk], f"ob{b}_{blk}", b, r0)
                else:
                    # tap-outer: weight-stationary runs of NBLK matmuls so
                    # consecutive LDWEIGHTS are identical (dedupable)
                    for it, (dy, dx) in enumerate(taps):
                        lhsT = wt[:, dy, dx, :]
                        for blk in range(NBLK):
                            r0 = blk * RB
                            nc.tensor.matmul(
                                out=pst[blk][:, :, :],
                                lhsT=lhsT,
                                rhs=xt[:, r0 + dy : r0 + dy + RB, dx : dx + W],
                                start=(it == 0),
                                stop=(it == len(taps) - 1),
                                skip_group_check=True,
                            )
                    for blk in range(NBLK):
                        evict(pst[blk], f"ob{b}_{blk}", b, blk * RB)
    nc.finalize()
    return nc


def _host_prep(x, routing_weight, weight, bias, mm_dtype):
    import ml_dtypes

    r = np.asarray(routing_weight, dtype=np.float32)
    # w_mix[b, c, kh, kw, oc] = sum_k r[b,k] * weight[k, oc, c, kh, kw]
    wflat = np.ascontiguousarray(np.transpose(weight, (0, 2, 3, 4, 1))).reshape(
        KEXP, -1
    )
    w_mix = (r @ wflat).reshape(BS, CIN, WSZ)
    b_mixT = np.ascontiguousarray((r @ bias).T)  # [oc, bs]

    np_in = ml_dtypes.bfloat16 if mm_dtype == "bf16" else np.float32
    pk = np.zeros((BS, CIN, PKSZ), dtype=np_in)
    pk[:, :, :WSZ] = w_mix
    xpad = pk[:, :, WSZ:].reshape(BS, CIN, HP, WP)
    xpad[:, :, 1 : H + 1, 1 : W + 1] = x

    in_maps = []
    for i in range(N_CORES):
        sl = slice(i * P, (i + 1) * P)
        in_maps.append(
            {
                "xw": np.ascontiguousarray(pk[sl]),
                "bm": np.ascontiguousarray(b_mixT[:, sl]),
            }
        )
    return in_maps


def _install_ntff_hook():
    """bass_utils imports antenv.axon_hooks for trace=True; the installed
    antenv lacks it. Provide it, registering the ctypes NTFF hook against
    libaxon_pjrt.so (same as trn_boot's _ntff_profile_via_ctypes)."""
    try:
        import antenv.axon_hooks  # noqa: F401

        return
    except ImportError:
        pass
    import contextlib
    import ctypes
    import sys as _sys
    import types

    hook = None
    so_path = "/opt/axon/libaxon_pjrt.so"
    if os.path.exists(so_path):
        lib = ctypes.CDLL(so_path)
        if hasattr(lib, "axon_start_nrt_profile"):
            lib.axon_start_nrt_profile.argtypes = [
                ctypes.POINTER(ctypes.c_int64),
                ctypes.c_size_t,
            ]
            lib.axon_start_nrt_profile.restype = ctypes.c_int64
            lib.axon_stop_nrt_profile.argtypes = [ctypes.c_char_p]
            lib.axon_stop_nrt_profile.restype = ctypes.c_int64

            @contextlib.contextmanager
            def _hook(output_dir, device_ids):
                import jax

                jax.devices()
                if device_ids:
                    ids = (ctypes.c_int64 * len(device_ids))(*device_ids)
                    rc = lib.axon_start_nrt_profile(ids, len(device_ids))
                else:
                    rc = lib.axon_start_nrt_profile(None, 0)
                if rc != 0:
                    raise RuntimeError(f"axon_start_nrt_profile rc={rc}")
                try:
                    yield
                finally:
                    n = lib.axon_stop_nrt_profile(str(output_dir).encode())
                    print(f"ntff profile: {n} file(s) -> {output_dir}")

            hook = _hook

    m = types.ModuleType("antenv.axon_hooks")
    m._hook = hook
    m.get_axon_ntff_profile_hook = lambda: m._hook

    def _set(h):
        m._hook = h

    m.set_axon_ntff_profile_hook = _set
    _sys.modules["antenv.axon_hooks"] = m


LDW_OPT = True


def _patch_ldw_opt():
    """Flip walrus's --enable-ldw-opt to true so consecutive identical
    LDWEIGHTS (tap-outer weight-stationary runs) get deduped."""
    import concourse.bass_utils as bu

    if getattr(bu, "_ldw_patched", False):
        return
    orig = bu.run_command

    def patched(argv, **kw):
        if isinstance(argv, list):
            argv = [
                "--enable-ldw-opt=true" if a == "--enable-ldw-opt=false" else a
                for a in argv
            ]
        return orig(argv, **kw)

    bu.run_command = patched
    bu._ldw_patched = True


def _run(in_maps, mm_dtype, **kw):
    _install_ntff_hook()
    if LDW_OPT:
        _patch_ldw_opt()
    from concourse.bass_utils import run_bass_kernel_spmd

    key = ("nc", mm_dtype)
    if key not in _CACHE:
        _CACHE[key] = _build_nc(mm_dtype)
    nc = _CACHE[key]
    return run_bass_kernel_spmd(nc, in_maps, list(range(N_CORES)), **kw)


def kernel(x, routing_weight, weight, bias):
    in_maps = _host_prep(x, routing_weight, weight, bias, MM_DTYPE)
    res = _run(in_maps, MM_DTYPE)
    return np.concatenate([res.results[i]["out"] for i in range(N_CORES)], axis=0)


# used by test.py for the profiled run
def kernel_profiled(x, routing_weight, weight, bias):
    in_maps = _host_prep(x, routing_weight, weight, bias, MM_DTYPE)
    res = _run(in_maps, MM_DTYPE, trace=True)
    out = np.concatenate([res.results[i]["out"] for i in range(N_CORES)], axis=0)
    return out, res
